# revision 31
# baseline (speedup 1.0000x reference)
"""Trainium2 Bass kernel for nn_BasicBlock (binary activation + binarized
weight-standardized 3x3 conv + residual + PReLU).

Contract: kernel(**inputs) takes FULL unsharded numpy inputs (keys as in
setup_inputs) and returns the FULL [32, 512, 28, 28] float32 output.
Internally shards the batch dim across 8 NeuronCores (4 images each); the
small conv weight + per-channel vectors are replicated.

Key math facts exploited:
- forward activations are sign(x*beta+b0) in {-1,0,1} and forward weights
  are sf[o]*gain[o]*sign(w_std) with sign in {-1,0,1}, so the conv
  contraction is exact in fp8 (products are +-1, fp32 PSUM accumulation);
  the per-channel scalar alpha*sf*gain folds into the epilogue.
- fp8e4 DoubleRow packs two contraction rows per PE cell (2 cin chunks
  per matmul), halving the matmul count.
- conv loop is n-outer, h2-inner: each (q,t) weight is reused for the 2
  spatial-half matmuls, so LDWEIGHTS (256 cols, ~213ns) hides under the
  ~2x166ns matmul pair and the stream paces at the matmul rate.
- epilogue: z = conv*alphabar + residual (DVE, drains PSUM bank),
  e = prelu(z + b1) (ACT Prelu with per-channel AP alpha),
  out = e + b2 (GpSimd add+mult form; the op1=bypass ucode path is 10x
  slower).
- ~90 junk identity matmuls at program start warm the PE HAM clock-gate
  (1.2 -> 2.4 GHz) while the weight DMA is in flight.
- per-channel vectors load as [128,4]-strided DMAs on the scalar HWDGE
  ring (cheap there; on the gpsimd SWDGE ring the Q7 descriptor loop
  takes ~20us for 128 scattered rows).
"""

import numpy as np

import concourse.bass as bass
import concourse.mybir as mybir
import concourse.tile as tile
from concourse import bacc
from concourse.masks import make_identity

# problem constants (hardcoded per harness contract)
N_CORES = 8
N_PER = 4          # images per core (32 / 8)
C = 512            # Cin == Cout
H = W = 28
HP = WP = 30       # zero-padded spatial
TAPS = 9
KFAN = C * TAPS    # 4608 = fan-in per output channel
ALPHA = 0.2
BETA = 1.0
EPS = 1e-5
WS_SCALE = 1.0 / float(np.sqrt(KFAN))  # fan_in**-0.5
NCH = C // 128     # 4 channel chunks of 128
NPAIR = NCH // 2   # 2 DoubleRow pairs of chunks
ROWS_PER_TILE = 14 # output rows per matmul tile
NSPAT = H // ROWS_PER_TILE  # 2 spatial tiles per image
ACT_IMG = 912  # padded 30x30 image (900) + 12 slack: %16==0 for DoubleRow
BIG = 1e30     # sign-via-clamp scale for the GpSimd xsign path

FP32 = mybir.dt.float32
BF16 = mybir.dt.bfloat16
FP8 = mybir.dt.float8e4


def build_program():
    nc = bacc.Bacc(
        "TRN2",
        target_bir_lowering=False,
        debug=False,
        num_devices=1,
        num_swdge_queues=4,
    )
    x_h = nc.declare_dram_parameter("x", [N_PER, C, H, W], FP32, isOutput=False)
    w_h = nc.declare_dram_parameter("conv_weight", [C, C, 3, 3], FP32, isOutput=False)
    gain_h = nc.declare_dram_parameter("gain", [C], FP32, isOutput=False)
    b0_h = nc.declare_dram_parameter("move0_bias", [C], FP32, isOutput=False)
    b1_h = nc.declare_dram_parameter("move1_bias", [C], FP32, isOutput=False)
    pa_h = nc.declare_dram_parameter("prelu_a", [C], FP32, isOutput=False)
    b2_h = nc.declare_dram_parameter("move2_bias", [C], FP32, isOutput=False)
    out_h = nc.declare_dram_parameter("out", [N_PER, C, H, W], FP32, isOutput=True)

    x_ap = x_h[:, :, :, :]
    w_ap = w_h[:, :, :, :]
    out_ap = out_h[:, :, :, :]

    with tile.TileContext(nc) as tc:
        with (
            tc.tile_pool(name="persist", bufs=1) as persist,
            tc.tile_pool(name="scratch", bufs=2) as scratch,
            tc.tile_pool(name="stats", bufs=4) as stats,
            tc.tile_pool(name="epi", bufs=4) as epi,
            tc.tile_pool(name="epi_o", bufs=8) as epi_o,
            tc.tile_pool(name="psum_mm", bufs=6, space="PSUM") as psum_mm,
            tc.tile_pool(name="psum_tr", bufs=2, space="PSUM") as psum_tr,
        ):
            w_flat = w_ap.rearrange("o i a b -> o (i a b)")
            w_tiles = []

            # per-channel vectors: scalar HWDGE ring, b0 first
            def load_vec(dram_h, name):
                t = persist.tile([128, NCH], FP32, tag=name, name=name)
                nc.scalar.dma_start(
                    out=t, in_=dram_h[:].rearrange("(c p) -> p c", p=128)
                )
                return [t[:, c : c + 1] for c in range(NCH)]

            b0_c = load_vec(b0_h, "b0")
            gain_c = load_vec(gain_h, "gain")
            b1_c = load_vec(b1_h, "b1")
            pa_c = load_vec(pa_h, "pa")
            b2_c = load_vec(b2_h, "b2")

            # w chunk0: 9 sg-aligned pieces on sync (stats pipeline per
            # piece); chunks 1,3 on sync; chunk 2 on scalar after vecs
            PIECES5 = [(0, 1024), (1024, 1024), (2048, 1024), (3072, 1024),
                       (4096, 512)]
            PIECES9 = [(j * 512, 512) for j in range(TAPS)]

            def w_dma(eng, m, pieces):
                wt = scratch.tile([128, KFAN], FP32, tag="wtile", name=f"wt{m}")
                for (c0, cw) in pieces:
                    eng.dma_start(
                        out=wt[:, c0 : c0 + cw],
                        in_=w_flat[m * 128 : (m + 1) * 128, c0 : c0 + cw],
                    )
                w_tiles.append(wt)

            w_dma(nc.sync, 0, PIECES9)

            # identity for transposes + HAM warmup (gpsimd, first)
            ident = persist.tile([128, 128], BF16, tag="ident")
            make_identity(nc, ident)

            # x: gpsimd SWDGE; image 0 in half-chunk pieces so chunk 0
            # lands first
            xs_all = persist.tile([128, NCH, N_PER, H, W], FP32, tag="xs", name="xs")
            xr = x_ap.rearrange("n (cc p) h w -> p cc n (h w)", p=128)
            for c in range(NCH):
                for hh in range(2):
                    r0 = hh * (H * W // 2)
                    nc.gpsimd.dma_start(
                        out=xs_all[:, c, 0].rearrange("p h w -> p (h w)")[
                            :, r0 : r0 + H * W // 2
                        ],
                        in_=xr[:, c, 0, r0 : r0 + H * W // 2],
                    )
            for n in range(1, N_PER):
                for ch in range(2):
                    nc.gpsimd.dma_start(
                        out=xs_all[:, 2 * ch : 2 * ch + 2, n].rearrange(
                            "p c h w -> p c (h w)"
                        ),
                        in_=xr[:, 2 * ch : 2 * ch + 2, n, :],
                    )

            w_dma(nc.sync, 1, PIECES5)
            w_dma(nc.scalar, 2, PIECES5)
            w_dma(nc.sync, 3, PIECES5)

            # act images + border memsets (n0/n1 on idle DVE, rest gpsimd)
            act_img = []
            for q in range(NPAIR):
                row = []
                for n in range(N_PER):
                    ap_t = persist.tile(
                        [128, 2, ACT_IMG], FP8, tag=f"act{q}_{n}", name=f"act{q}_{n}"
                    )
                    row.append(ap_t)
                act_img.append(row)

            def act_memset(q, n, e):
                ap_t = act_img[q][n]
                e.memset(ap_t[:, :, 0:WP], 0.0)
                e.memset(ap_t[:, :, 29 * WP : ACT_IMG], 0.0)
                mid = ap_t[:, :, WP : 29 * WP].rearrange("p h (r c) -> p h r c", c=WP)
                e.memset(mid[:, :, :, 0:1], 0.0)
                e.memset(mid[:, :, :, 29:30], 0.0)

            for q in range(NPAIR):
                act_memset(q, 0, nc.vector)
                act_memset(q, 1, nc.vector)

            # ---- weight prep helpers -------------------------------------
            lhsT = persist.tile(
                [128, TAPS, NPAIR, 2, C], FP8, tag="lhsT", name="lhsT"
            )
            alphabar = {}
            wsigns = {}
            mvs = {}

            def weight_stats(m):
                wt = w_tiles[m]
                st = stats.tile([128, TAPS, 6], FP32, tag="bnst", name="bnst")
                wt3 = wt.rearrange("p (a b) -> p a b", b=512)
                for sg in range(TAPS):
                    nc.vector.bn_stats(out=st[:, sg, :], in_=wt3[:, sg, :])
                mv = stats.tile([128, 2], FP32, tag="bnagg", name="bnagg")
                nc.vector.bn_aggr(out=mv, in_=st)
                negmean = stats.tile([128, 1], FP32, tag="negmean", name="negmean")
                nc.vector.tensor_scalar_mul(out=negmean, in0=mv[:, 0:1], scalar1=-1.0)
                mvs[m] = [mv, negmean, None]

            def wsign_block(m, b):
                if b == 0:
                    wsigns[m] = scratch.tile(
                        [128, KFAN], BF16, tag="wsign", name=f"ws{m}"
                    )
                nc.scalar.activation(
                    out=wsigns[m][:, b * 1152 : (b + 1) * 1152],
                    in_=w_tiles[m][:, b * 1152 : (b + 1) * 1152],
                    func=mybir.ActivationFunctionType.Sign,
                    bias=mvs[m][1],
                )

            def stdeps_sqrt(m):
                sd = stats.tile([128, 1], FP32, tag="stdeps", name=f"sd{m}")
                nc.scalar.activation(
                    out=sd, in_=mvs[m][0][:, 1:2],
                    func=mybir.ActivationFunctionType.Sqrt,
                )
                mvs[m][2] = sd

            def wabs(m, sumabs_t):
                wt = w_tiles[m]
                for b in range(NCH):
                    nc.scalar.activation(
                        out=wt[:, b * 1152 : (b + 1) * 1152],
                        in_=wt[:, b * 1152 : (b + 1) * 1152],
                        func=mybir.ActivationFunctionType.Abs,
                        bias=mvs[m][1],
                        accum_out=sumabs_t[:, b : b + 1],
                    )

            def alphabar_tail(m, sumabs_t):
                sde = stats.tile([128, 1], FP32, tag="sde", name=f"sde{m}")
                nc.vector.tensor_scalar_add(out=sde, in0=mvs[m][2], scalar1=EPS)
                inv = stats.tile([128, 1], FP32, tag="inv", name=f"inv{m}")
                nc.vector.reciprocal(out=inv, in_=sde)
                s1 = stats.tile([128, 1], FP32, tag="s1", name=f"s1{m}")
                nc.vector.tensor_reduce(
                    out=s1, in_=sumabs_t[:, 0:NCH], axis=mybir.AxisListType.X,
                    op=mybir.AluOpType.add,
                )
                ab = persist.tile(
                    [128, 1], FP32, tag=f"alphabar{m}", name=f"alphabar{m}"
                )
                nc.vector.tensor_tensor(
                    out=ab, in0=s1, in1=inv, op=mybir.AluOpType.mult
                )
                nc.vector.tensor_tensor(
                    out=ab, in0=ab, in1=gain_c[m], op=mybir.AluOpType.mult
                )
                nc.vector.tensor_scalar_mul(
                    out=ab, in0=ab, scalar1=ALPHA * WS_SCALE / KFAN
                )
                alphabar[m] = ab

            def transpose_cast_tap(m, t):
                # baseline style: 4 block transposes of tap t share one
                # PSUM tile -> single batched DVE cast
                ws3 = wsigns[m].rearrange("p (i t) -> p i t", t=TAPS)
                ps = psum_tr.tile([128, NCH * 128], BF16, tag="ptr", name="ptr")
                for b in range(NCH):
                    nc.tensor.transpose(
                        ps[:, b * 128 : (b + 1) * 128],
                        ws3[:, b * 128 : (b + 1) * 128, t],
                        ident,
                    )
                nc.vector.tensor_copy(
                    out=lhsT[:, t, :, :, m * 128 : (m + 1) * 128], in_=ps
                )

            # ---- xsign paths ---------------------------------------------
            def xsign_act(n, c):
                dst = act_img[c // 2][n][:, c % 2, : HP * WP].rearrange(
                    "p (h w) -> p h w", w=WP
                )[:, 1 : 1 + H, 1 : 1 + W]
                nc.scalar.activation(
                    out=dst,
                    in_=xs_all[:, c, n],
                    func=mybir.ActivationFunctionType.Sign,
                    bias=b0_c[c],
                    scale=BETA,
                )

            def xsign_gps(n, c):
                dst = act_img[c // 2][n][:, c % 2, : HP * WP].rearrange(
                    "p (h w) -> p h w", w=WP
                )[:, 1 : 1 + H, 1 : 1 + W]
                tmp = epi.tile([128, H, W], FP32, tag="xtmp", name="xtmp")
                nc.gpsimd.tensor_scalar(
                    out=tmp, in0=xs_all[:, c, n], scalar1=b0_c[c], scalar2=BIG,
                    op0=mybir.AluOpType.add, op1=mybir.AluOpType.mult,
                )
                nc.gpsimd.tensor_scalar(
                    out=dst, in0=tmp, scalar1=1.0, scalar2=-1.0,
                    op0=mybir.AluOpType.min, op1=mybir.AluOpType.max,
                )

            # ---- HAM warmup ----------------------------------------------
            jp = psum_mm.tile(
                [128, ROWS_PER_TILE * W], FP32, tag="acc", name="junk"
            )
            for _ in range(90):
                nc.tensor.matmul(jp[:, :128], ident, ident, start=True, stop=True)

            # ---- prologue chain for chunk 0 ------------------------------
            weight_stats(0)
            wsign_block(0, 0)
            wsign_block(0, 1)
            xsign_act(0, 0)
            xsign_act(0, 1)
            wsign_block(0, 2)
            wsign_block(0, 3)
            stdeps_sqrt(0)
            for t in range(TAPS):
                transpose_cast_tap(0, t)
            xsign_gps(0, 2)
            xsign_gps(0, 3)

            for q in range(NPAIR):
                act_memset(q, 2, nc.gpsimd)
                act_memset(q, 3, nc.gpsimd)

            xsign_act(1, 0)
            xsign_act(1, 1)
            xsign_gps(1, 2)
            xsign_gps(1, 3)

            sumabs0 = stats.tile([128, NCH], FP32, tag="sumabs", name="sumabs0")
            wabs(0, sumabs0)
            alphabar_tail(0, sumabs0)

            xsign_act(2, 0)
            xsign_act(2, 1)
            xsign_gps(2, 2)
            xsign_gps(2, 3)
            xsign_act(3, 0)
            xsign_act(3, 1)
            xsign_gps(3, 2)
            xsign_gps(3, 3)

            # ---- conv ----------------------------------------------------
            def prep_act(m):
                for b in range(NCH):
                    wsign_block(m, b)
                stdeps_sqrt(m)
                sa = stats.tile([128, NCH], FP32, tag="sumabs", name=f"sa{m}")
                wabs(m, sa)
                alphabar_tail(m, sa)

            def prep_b(m):
                for t in range(TAPS):
                    transpose_cast_tap(m, t)

            def conv_group(m, n):
                accs = []
                for h2 in range(NSPAT):
                    accs.append(
                        psum_mm.tile(
                            [128, ROWS_PER_TILE * W], FP32, tag="acc", name="acc"
                        )
                    )
                i = 0
                for q in range(NPAIR):
                    av = act_img[q][n][:, :, : HP * WP].rearrange(
                        "p h (r c) -> p h r c", c=WP
                    )
                    for t in range(TAPS):
                        dy, dx = t // 3, t % 3
                        for h2 in range(NSPAT):
                            y0 = h2 * ROWS_PER_TILE
                            rhs = av[
                                :, :, y0 + dy : y0 + dy + ROWS_PER_TILE,
                                dx : dx + W,
                            ]
                            nc.tensor.matmul(
                                accs[h2],
                                lhsT[:, t, q, :, m * 128 : (m + 1) * 128],
                                rhs,
                                start=(i == 0),
                                stop=(i == NPAIR * TAPS - 1),
                                perf_mode=mybir.MatmulPerfMode.DoubleRow,
                            )
                        i += 1
                for h2 in range(NSPAT):
                    y0 = h2 * ROWS_PER_TILE
                    accv = accs[h2].rearrange("p (h w) -> p h w", w=W)
                    res = xs_all[:, m, n, y0 : y0 + ROWS_PER_TILE, :]
                    z = epi.tile([128, ROWS_PER_TILE, W], FP32, tag="z", name="z")
                    nc.vector.scalar_tensor_tensor(
                        out=z, in0=accv, scalar=alphabar[m], in1=res,
                        op0=mybir.AluOpType.mult, op1=mybir.AluOpType.add,
                    )
                    e = epi.tile([128, ROWS_PER_TILE, W], FP32, tag="e", name="e")
                    nc.scalar.activation(
                        out=e, in_=z,
                        func=mybir.ActivationFunctionType.Prelu,
                        bias=b1_c[m], alpha=pa_c[m],
                    )
                    o = epi_o.tile(
                        [128, ROWS_PER_TILE, W], FP32, tag="oo", name="oo"
                    )
                    nc.gpsimd.tensor_scalar(
                        out=o, in0=e, scalar1=b2_c[m], scalar2=1.0,
                        op0=mybir.AluOpType.add, op1=mybir.AluOpType.mult,
                    )
                    nsplit = 2 if (m == NCH - 1 and n == N_PER - 1) else 1
                    rr = ROWS_PER_TILE // nsplit
                    for s in range(nsplit):
                        nc.sync.dma_start(
                            out=out_ap[
                                n, m * 128 : (m + 1) * 128,
                                y0 + s * rr : y0 + (s + 1) * rr, :,
                            ],
                            in_=o[:, s * rr : (s + 1) * rr],
                        )

            for m in range(NCH):
                if m + 1 < NCH:
                    weight_stats(m + 1)
                for n in range(N_PER):
                    conv_group(m, n)
                    if m + 1 < NCH:
                        if n == 0:
                            prep_act(m + 1)
                        elif n == 1:
                            prep_b(m + 1)

    nc.finalize()
    return nc


_NC_CACHE = None


def _get_program():
    global _NC_CACHE
    if _NC_CACHE is None:
        _NC_CACHE = build_program()
    return _NC_CACHE


def kernel(**inputs):
    from concourse.bass_utils import run_bass_kernel_spmd

    x = np.ascontiguousarray(np.asarray(inputs["x"], dtype=np.float32))
    shared = {
        name: np.ascontiguousarray(np.asarray(inputs[name], dtype=np.float32))
        for name in (
            "conv_weight", "gain", "move0_bias", "move1_bias", "prelu_a",
            "move2_bias",
        )
    }
    nc = _get_program()
    in_maps = [
        {"x": x[i * N_PER : (i + 1) * N_PER], **shared} for i in range(N_CORES)
    ]
    res = run_bass_kernel_spmd(nc, in_maps, core_ids=list(range(N_CORES)))
    return np.concatenate([r["out"] for r in res.results], axis=0)
